# revision 11
# baseline (speedup 1.0000x reference)
"""Trainium2 Bass kernel for the additive-attention problem.

reference math:
    rec[b,h]    = sum_r rnn_state[b,r] * W_rec[h,r]
    scores[t,b] = sum_h tanh(enc[t,b,h] + rec[b,h]) * w_score[h] + b_score + mask[t,b]
    out         = softmax(scores, axis=t)          # (T, B) float32

Sharding: data-parallel over B across 8 cores (BL=4 batch columns per core).
Softmax is over T (core-local), so no collectives.

Design (h-major layout, enc pre-staged on host as bf16, v3):
  - host stages enc as [granule=256 t-rows][p=h%128][hc][b][t] bf16; tiles
    are 1-2 granules (schedule 256,256,512x7) -> 1MB DMAs, small first tiles
    for fast pipeline fill, few big tanh instrs in steady state (ScalarE
    ACTIVATE pays ~354ns fixed per instr + ~0.98ns/elem).
  - rec computed on device (16 small bf16 matmuls); in h-major layout rec is
    a per-partition scalar per (hc,b) slice -> VectorE tensor_scalar_add.
  - tanh: ONE ScalarE activation per tile (bf16), the kernel bottleneck
    (~64us of ACTIVATE); activation table pre-warmed with a dummy tanh so
    the ~1.5us ACT_TABLE_LOAD overlaps the first DMAs.
  - mask is pre-added INTO the PSUM scores tile by an identity-matmul before
    the main loop (start=True); all score matmuls then accumulate onto it
    (start=False, per-element has_written semantics).
  - score reduction over h on TensorE: per 128 consecutive t (fixed b),
    lhsT = tanh-slice (p=h, m=t) stationary, rhs = w chunk (128,1) bf16,
    accumulated over 4 h-chunks into one column of the persistent PSUM
    scores tile (p=t%128, f=(t//128, b)).
  - epilogue: ScalarE exp (PSUM src) -> one matmul with rhs=[I|ones] doing
    transpose AND row sums together -> m4 block-mask matmul broadcasts
    per-b totals -> reciprocal -> scale -> output DMA as (BL,T) 512B runs.
b_score cancels in softmax and is ignored.  No max-subtraction needed:
|scores| <= ||w_score||_1 <~ 25, safely inside f32 exp range.
"""

import numpy as np

T, B, H, R = 4096, 32, 512, 512
NCORES = 8
BL = B // NCORES          # 4 local batch columns
GT = 256                  # granule t rows (host staging unit)
NG = T // GT              # 16 granules
HC = H // 128             # 4 h-chunks
RC = R // 128             # 4 r-chunks
# tiles in granules: small first tiles for pipeline fill, 512-row steady
TSCHED = [1, 1, 2, 2, 2, 2, 2, 2, 1, 1]
assert sum(TSCHED) == NG

_GRAPH = None


def _build_graph():
    import concourse.bass as bass
    import concourse.tile as tile
    from concourse import bacc, mybir
    from concourse.masks import make_identity

    f32 = mybir.dt.float32
    bf16 = mybir.dt.bfloat16
    nc = bacc.Bacc()

    encT = nc.declare_dram_parameter(
        "encT", [NG, 128, HC, BL, GT], bf16, isOutput=False
    )
    maskd = nc.declare_dram_parameter("maskd", [T, BL], f32, isOutput=False)
    rnnT = nc.declare_dram_parameter("rnnT", [RC, 128, BL], bf16, isOutput=False)
    wrecT = nc.declare_dram_parameter("wrecT", [RC, 128, H], bf16, isOutput=False)
    wT = nc.declare_dram_parameter("wT", [128, HC], bf16, isOutput=False)
    m4d = nc.declare_dram_parameter("m4", [128, 128], f32, isOutput=False)
    out = nc.declare_dram_parameter("out", [BL, T], f32, isOutput=True)

    with tile.TileContext(nc) as tc:
        with (
            tc.tile_pool(name="singles", bufs=1) as singles,
            tc.tile_pool(name="xpool", bufs=4) as xpool,
            tc.tile_pool(name="ypool", bufs=2) as ypool,
            tc.tile_pool(name="spsum", bufs=1, space="PSUM") as spsum,
            tc.tile_pool(name="epsum", bufs=2, space="PSUM") as epsum,
        ):
            # ---------- enc granule DMAs on the sync HWDGE ring ----------
            # (issued first so the SDMA engines start on tile 0 immediately;
            # small inputs go on the gpsimd SWDGE ring in parallel)
            encv = encT.rearrange("g p c b t -> g p c b t")

            # identity+ones for the combined transpose/row-sum matmul, and
            # an early dummy tanh to pull ACT_TABLE_LOAD off the critical path
            idext = singles.tile([128, 129], f32)
            make_identity(nc, idext[:, 0:128])
            nc.gpsimd.memset(idext[:, 128:129], 1.0)
            warm = singles.tile([128, 1], f32)
            nc.scalar.activation(
                out=warm[:], in_=idext[:, 0:1],
                func=mybir.ActivationFunctionType.Tanh,
            )

            # rec-chain inputs on the sync HWDGE ring, issued BEFORE the enc
            # granules; concurrent DMAs fair-share SDMA bandwidth, so granules
            # 1+ are gated on this load (add_dep below) to keep the rec chain
            # and granule 0 from being starved by the prefetch burst.
            rnn_sb = singles.tile([128, RC, BL], bf16)
            nc.sync.dma_start(out=rnn_sb[:], in_=rnnT.rearrange("r p b -> p r b"))
            wrec_sb = singles.tile([128, RC, H], bf16)
            wrec_dma = nc.sync.dma_start(
                out=wrec_sb[:], in_=wrecT.rearrange("r p h -> p r h")
            )
            # tile0's granule goes on the wire right behind the rec inputs,
            # ahead of the other small loads
            X0 = xpool.tile([128, TSCHED[0], HC, BL, GT], bf16)
            for g in range(TSCHED[0]):
                nc.sync.dma_start(out=X0[:, g], in_=encv[g])

            w_sb = singles.tile([128, HC], bf16)
            nc.sync.dma_start(out=w_sb[:], in_=wT[:])
            mask_sb = singles.tile([128, NG * (GT // 128), BL], f32)
            nc.sync.dma_start(
                out=mask_sb[:], in_=maskd.rearrange("(a p) b -> p a b", p=128)
            )
            m4 = singles.tile([128, 128], f32)
            nc.sync.dma_start(out=m4[:], in_=m4d[:])

            # ---------- rec[h, b] = sum_r W_rec[h,r] rnn[b,r] ----------
            rec_ps = epsum.tile([128, HC, BL], f32, tag="epi")
            for hc in range(HC):
                for rc in range(RC):
                    nc.tensor.matmul(
                        rec_ps[:, hc, :],
                        lhsT=wrec_sb[:, rc, hc * 128 : (hc + 1) * 128],
                        rhs=rnn_sb[:, rc, :],
                        start=(rc == 0),
                        stop=(rc == RC - 1),
                    )
            rec_sb = singles.tile([128, HC, BL], f32)
            nc.vector.tensor_copy(out=rec_sb[:], in_=rec_ps[:])

            # persistent scores accumulator: (p=t%128, f=(t//128, b));
            # seeded with the additive mask (identity matmul, start=True) so
            # every score matmul just accumulates (start=False).
            scores_ps = spsum.tile([128, NG * (GT // 128) * BL], f32)
            nc.tensor.matmul(
                scores_ps[:],
                lhsT=idext[:, 0:128],
                rhs=mask_sb[:].rearrange("p a b -> p (a b)"),
                start=True,
                stop=False,
                skip_group_check=True,
            )

            # ---------- main loop over t tiles ----------
            from concourse.tile_rust import add_dep_helper

            g0 = 0
            for k, ng in enumerate(TSCHED):
                if k == 0:
                    X = X0
                else:
                    X = xpool.tile([128, ng, HC, BL, GT], bf16)
                    for g in range(ng):
                        d = nc.sync.dma_start(out=X[:, g], in_=encv[g0 + g])
                        if g0 + g <= 4:
                            add_dep_helper(
                                d.ins, wrec_dma.ins, sync=True,
                                reason="keep startup burst off the rec chain",
                            )
                for hc in range(HC):
                    for b in range(BL):
                        nc.vector.tensor_scalar_add(
                            out=X[:, :, hc, b, :],
                            in0=X[:, :, hc, b, :],
                            scalar1=rec_sb[:, hc, b : b + 1],
                        )
                Y = ypool.tile([128, ng, HC, BL, GT], bf16)
                nc.scalar.activation(
                    out=Y[:],
                    in_=X[:],
                    func=mybir.ActivationFunctionType.Tanh,
                )
                for g in range(ng):
                    for ts in range(GT // 128):
                        a = (g0 + g) * (GT // 128) + ts
                        for b in range(BL):
                            c = a * BL + b
                            for hc in range(HC):
                                nc.tensor.matmul(
                                    scores_ps[:, c : c + 1],
                                    lhsT=Y[:, g, hc, b, ts * 128 : (ts + 1) * 128],
                                    rhs=w_sb[:, hc : hc + 1],
                                    start=False,
                                    stop=(hc == HC - 1),
                                    skip_group_check=True,
                                )
                g0 += ng

            # ---------- exp, transpose+row-sums, normalize, output ----------
            E = singles.tile([128, 128], f32)
            nc.scalar.activation(
                out=E[:], in_=scores_ps[:], func=mybir.ActivationFunctionType.Exp
            )
            # one matmul: cols 0..127 = E^T (p=(a,b), f=t%128), col 128 = row sums
            attx = epsum.tile([128, 129], f32, tag="epi")
            nc.tensor.matmul(
                attx[:], lhsT=E[:], rhs=idext[:], start=True, stop=True
            )
            rs_sb = singles.tile([128, 1], f32)
            nc.vector.tensor_copy(out=rs_sb[:], in_=attx[:, 128:129])
            denom = epsum.tile([128, 1], f32, tag="epi")
            nc.tensor.matmul(
                denom[:], lhsT=m4[:], rhs=rs_sb[:], start=True, stop=True
            )
            recip = singles.tile([128, 1], f32)
            nc.vector.reciprocal(out=recip[:], in_=denom[:])
            att_out = singles.tile([128, 128], f32)
            nc.vector.tensor_scalar_mul(
                out=att_out[:], in0=attx[:, 0:128], scalar1=recip[:]
            )
            # partition p = (a=t//128, b) holds 128 contiguous t values for col b
            nc.sync.dma_start(
                out=out.rearrange("b (a tp) -> a b tp", tp=128),
                in_=att_out[:],
            )

    nc.compile()
    return nc


def _get_graph():
    global _GRAPH
    if _GRAPH is None:
        _GRAPH = _build_graph()
    return _GRAPH


def make_in_maps(enc, mask, rnn_state, W_rec, w_score):
    import ml_dtypes

    bf = ml_dtypes.bfloat16
    enc_bf = np.asarray(enc, dtype=np.float32).astype(bf)
    # [g, t, core, b, hc, p] view -> per-core [g, p, hc, b, t]
    e6 = enc_bf.reshape(NG, GT, NCORES, BL, HC, 128).transpose(2, 0, 5, 4, 3, 1)
    wrecT = np.ascontiguousarray(
        W_rec.T.astype(np.float32).astype(bf).reshape(RC, 128, H)
    )
    wTh = np.ascontiguousarray(
        w_score.astype(np.float32).astype(bf).reshape(HC, 128).T
    )
    m4 = (np.arange(128)[:, None] % BL == np.arange(128)[None, :] % BL).astype(
        np.float32
    )
    mask = np.asarray(mask, dtype=np.float32)
    rnn = np.asarray(rnn_state, dtype=np.float32)
    in_maps = []
    for c in range(NCORES):
        sl = slice(c * BL, (c + 1) * BL)
        in_maps.append(
            {
                "encT": np.ascontiguousarray(e6[c]),
                "maskd": np.ascontiguousarray(mask[:, sl]),
                "rnnT": np.ascontiguousarray(
                    rnn[sl].T.astype(bf).reshape(RC, 128, BL)
                ),
                "wrecT": wrecT,
                "wT": wTh,
                "m4": m4,
            }
        )
    return in_maps


def kernel(
    encoded_contribution,
    mask,
    rnn_state,
    prev_att_weights,
    W_rec,
    w_score,
    b_score,
):
    from concourse.bass_utils import run_bass_kernel_spmd

    nc = _get_graph()
    in_maps = make_in_maps(
        np.asarray(encoded_contribution),
        np.asarray(mask),
        np.asarray(rnn_state),
        np.asarray(W_rec),
        np.asarray(w_score),
    )
    res = run_bass_kernel_spmd(nc, in_maps, list(range(NCORES)))
    outs = [np.asarray(res.results[c]["out"]) for c in range(NCORES)]
    return np.concatenate([o.T for o in outs], axis=1).astype(np.float32)


# revision 18
# speedup vs baseline: 1.0376x; 1.0376x over previous
"""Trainium2 Bass kernel for the additive-attention problem.

reference math:
    rec[b,h]    = sum_r rnn_state[b,r] * W_rec[h,r]
    scores[t,b] = sum_h tanh(enc[t,b,h] + rec[b,h]) * w_score[h] + b_score + mask[t,b]
    out         = softmax(scores, axis=t)          # (T, B) float32

Sharding: data-parallel over B across 8 cores (BL=4 batch columns per core).
Softmax is over T (core-local), so no collectives.

Design (h-major layout, enc pre-staged on host as bf16, v3):
  - host stages enc as [granule=256 t-rows][p=h%128][hc][b][t] bf16; tiles
    are 1-2 granules (schedule 256,256,512x7) -> 1MB DMAs, small first tiles
    for fast pipeline fill, few big tanh instrs in steady state (ScalarE
    ACTIVATE pays ~354ns fixed per instr + ~0.98ns/elem).
  - rec computed on device (16 small bf16 matmuls); in h-major layout rec is
    a per-partition scalar per (hc,b) slice -> VectorE tensor_scalar_add.
  - tanh: ONE ScalarE activation per tile (bf16), the kernel bottleneck
    (~64us of ACTIVATE); activation table pre-warmed with a dummy tanh so
    the ~1.5us ACT_TABLE_LOAD overlaps the first DMAs.
  - mask is pre-added INTO the PSUM scores tile by an identity-matmul before
    the main loop (start=True); all score matmuls then accumulate onto it
    (start=False, per-element has_written semantics).
  - score reduction over h on TensorE: per 128 consecutive t (fixed b),
    lhsT = tanh-slice (p=h, m=t) stationary, rhs = w chunk (128,1) bf16,
    accumulated over 4 h-chunks into one column of the persistent PSUM
    scores tile (p=t%128, f=(t//128, b)).
  - epilogue: ScalarE exp (PSUM src) -> one matmul with rhs=[I|ones] doing
    transpose AND row sums together -> m4 block-mask matmul broadcasts
    per-b totals -> reciprocal -> scale -> output DMA as (BL,T) 512B runs.
b_score cancels in softmax and is ignored.  No max-subtraction needed:
|scores| <= ||w_score||_1 <~ 25, safely inside f32 exp range.
"""

import numpy as np

T, B, H, R = 4096, 32, 512, 512
NCORES = 8
BL = B // NCORES          # 4 local batch columns
GT = 256                  # granule t rows (host staging unit)
NG = T // GT              # 16 granules
HC = H // 128             # 4 h-chunks
RC = R // 128             # 4 r-chunks
# tiles in granules: small first tiles for pipeline fill, 512-row steady
TSCHED = [1, 1, 1, 1, 2, 2, 2, 2, 2, 1, 1]
assert sum(TSCHED) == NG

_GRAPH = None


def _build_graph():
    import concourse.bass as bass
    import concourse.tile as tile
    from concourse import bacc, mybir
    from concourse.masks import make_identity

    f32 = mybir.dt.float32
    bf16 = mybir.dt.bfloat16
    nc = bacc.Bacc()

    encT = nc.declare_dram_parameter(
        "encT", [NG, 128, HC, BL, GT], bf16, isOutput=False
    )
    rnnT = nc.declare_dram_parameter("rnnT", [RC, 128, BL], bf16, isOutput=False)
    wrecT = nc.declare_dram_parameter("wrecT", [RC, 128, H], bf16, isOutput=False)
    # packed small inputs: cols 0:128 m4, 128:256 mask (p=t%128, f=(a,b)),
    # 256:260 w_score chunks as f32 -> ONE small DMA on the sync ring
    packd = nc.declare_dram_parameter("packd", [128, 260], f32, isOutput=False)
    out = nc.declare_dram_parameter("out", [BL, T], f32, isOutput=True)

    with tile.TileContext(nc) as tc:
        with (
            tc.tile_pool(name="singles", bufs=1) as singles,
            tc.tile_pool(name="xpool", bufs=4) as xpool,
            tc.tile_pool(name="ypool", bufs=2) as ypool,
            tc.tile_pool(name="spsum", bufs=1, space="PSUM") as spsum,
            tc.tile_pool(name="epsum", bufs=2, space="PSUM") as epsum,
        ):
            # ---------- enc granule DMAs on the sync HWDGE ring ----------
            # (issued first so the SDMA engines start on tile 0 immediately;
            # small inputs go on the gpsimd SWDGE ring in parallel)
            encv = encT.rearrange("g p c b t -> g p c b t")

            # identity+ones for the combined transpose/row-sum matmul, and
            # an early dummy tanh to pull ACT_TABLE_LOAD off the critical path
            idext = singles.tile([128, 129], f32)
            make_identity(nc, idext[:, 0:128])
            nc.gpsimd.memset(idext[:, 128:129], 1.0)
            warm = singles.tile([128, 1], f32)
            nc.scalar.activation(
                out=warm[:], in_=idext[:, 0:1],
                func=mybir.ActivationFunctionType.Tanh,
            )

            # rec-chain inputs on the sync HWDGE ring, issued BEFORE the enc
            # granules; concurrent DMAs fair-share SDMA bandwidth, so granules
            # 1+ are gated on this load (add_dep below) to keep the rec chain
            # and granule 0 from being starved by the prefetch burst.
            rnn_sb = singles.tile([128, RC, BL], bf16)
            nc.sync.dma_start(out=rnn_sb[:], in_=rnnT.rearrange("r p b -> p r b"))
            wrec_sb = singles.tile([128, RC, H], bf16)
            wrec_dma = nc.sync.dma_start(
                out=wrec_sb[:], in_=wrecT.rearrange("r p h -> p r h")
            )
            # tile0's granule goes on the wire right behind the rec inputs,
            # ahead of the other small loads
            X0 = xpool.tile([128, TSCHED[0], HC, BL, GT], bf16)
            for g in range(TSCHED[0]):
                nc.sync.dma_start(out=X0[:, g], in_=encv[g])

            pack = singles.tile([128, 260], f32)
            nc.sync.dma_start(out=pack[:], in_=packd[:])
            m4 = pack[:, 0:128]
            mask_flat = pack[:, 128:256]
            w_sb = singles.tile([128, HC], bf16)
            nc.vector.tensor_copy(out=w_sb[:], in_=pack[:, 256:260])

            # ---------- rec[h, b] = sum_r W_rec[h,r] rnn[b,r] ----------
            rec_ps = epsum.tile([128, HC, BL], f32, tag="epi")
            for hc in range(HC):
                for rc in range(RC):
                    nc.tensor.matmul(
                        rec_ps[:, hc, :],
                        lhsT=wrec_sb[:, rc, hc * 128 : (hc + 1) * 128],
                        rhs=rnn_sb[:, rc, :],
                        start=(rc == 0),
                        stop=(rc == RC - 1),
                    )
            rec_sb = singles.tile([128, HC, BL], f32)
            nc.vector.tensor_copy(out=rec_sb[:], in_=rec_ps[:])

            # persistent scores accumulator: (p=t%128, f=(t//128, b));
            # seeded with the additive mask (identity matmul, start=True) so
            # every score matmul just accumulates (start=False).
            scores_ps = spsum.tile([128, NG * (GT // 128) * BL], f32)
            nc.tensor.matmul(
                scores_ps[:],
                lhsT=idext[:, 0:128],
                rhs=mask_flat,
                start=True,
                stop=False,
                skip_group_check=True,
            )

            # ---------- main loop over t tiles ----------
            from concourse.tile_rust import add_dep_helper

            g0 = 0
            for k, ng in enumerate(TSCHED):
                if k == 0:
                    X = X0
                else:
                    X = xpool.tile([128, ng, HC, BL, GT], bf16)
                    for g in range(ng):
                        d = nc.sync.dma_start(out=X[:, g], in_=encv[g0 + g])
                        if g0 + g <= 6:
                            add_dep_helper(
                                d.ins, wrec_dma.ins, sync=True,
                                reason="keep startup burst off the rec chain",
                            )
                for hc in range(HC):
                    for b in range(BL):
                        nc.vector.tensor_scalar_add(
                            out=X[:, :, hc, b, :],
                            in0=X[:, :, hc, b, :],
                            scalar1=rec_sb[:, hc, b : b + 1],
                        )
                Y = ypool.tile([128, ng, HC, BL, GT], bf16)
                nc.scalar.activation(
                    out=Y[:],
                    in_=X[:],
                    func=mybir.ActivationFunctionType.Tanh,
                )
                for g in range(ng):
                    for ts in range(GT // 128):
                        a = (g0 + g) * (GT // 128) + ts
                        for b in range(BL):
                            c = a * BL + b
                            for hc in range(HC):
                                nc.tensor.matmul(
                                    scores_ps[:, c : c + 1],
                                    lhsT=Y[:, g, hc, b, ts * 128 : (ts + 1) * 128],
                                    rhs=w_sb[:, hc : hc + 1],
                                    start=False,
                                    stop=(hc == HC - 1),
                                    skip_group_check=True,
                                )
                g0 += ng

            # ---------- exp, transpose+row-sums, normalize, output ----------
            E = singles.tile([128, 128], f32)
            nc.scalar.activation(
                out=E[:], in_=scores_ps[:], func=mybir.ActivationFunctionType.Exp
            )
            # one matmul: cols 0..127 = E^T (p=(a,b), f=t%128), col 128 = row sums
            attx = epsum.tile([128, 129], f32, tag="epi")
            nc.tensor.matmul(
                attx[:], lhsT=E[:], rhs=idext[:], start=True, stop=True
            )
            rs_sb = singles.tile([128, 1], f32)
            nc.vector.tensor_copy(out=rs_sb[:], in_=attx[:, 128:129])
            denom = epsum.tile([128, 1], f32, tag="epi")
            nc.tensor.matmul(
                denom[:], lhsT=m4, rhs=rs_sb[:], start=True, stop=True
            )
            recip = singles.tile([128, 1], f32)
            nc.vector.reciprocal(out=recip[:], in_=denom[:])
            att_out = singles.tile([128, 128], f32)
            nc.vector.tensor_scalar_mul(
                out=att_out[:], in0=attx[:, 0:128], scalar1=recip[:]
            )
            # partition p = (a=t//128, b) holds 128 contiguous t values for col b
            nc.sync.dma_start(
                out=out.rearrange("b (a tp) -> a b tp", tp=128),
                in_=att_out[:],
            )

    nc.compile()
    return nc


def _get_graph():
    global _GRAPH
    if _GRAPH is None:
        _GRAPH = _build_graph()
    return _GRAPH


def make_in_maps(enc, mask, rnn_state, W_rec, w_score):
    import ml_dtypes

    bf = ml_dtypes.bfloat16
    enc_bf = np.asarray(enc, dtype=np.float32).astype(bf)
    # [g, t, core, b, hc, p] view -> per-core [g, p, hc, b, t]
    e6 = enc_bf.reshape(NG, GT, NCORES, BL, HC, 128).transpose(2, 0, 5, 4, 3, 1)
    wrecT = np.ascontiguousarray(
        W_rec.T.astype(np.float32).astype(bf).reshape(RC, 128, H)
    )
    m4 = (np.arange(128)[:, None] % BL == np.arange(128)[None, :] % BL).astype(
        np.float32
    )
    mask = np.asarray(mask, dtype=np.float32)
    rnn = np.asarray(rnn_state, dtype=np.float32)
    wf = w_score.astype(np.float32).reshape(HC, 128).T  # (128, HC)
    in_maps = []
    for c in range(NCORES):
        sl = slice(c * BL, (c + 1) * BL)
        # mask in (p=t%128, f=(a=t//128, b)) layout, flattened to 128 cols
        mcols = np.ascontiguousarray(
            mask[:, sl].reshape(NG * (GT // 128), 128, BL).transpose(1, 0, 2)
        ).reshape(128, 128)
        packd = np.concatenate([m4, mcols, wf], axis=1).astype(np.float32)
        in_maps.append(
            {
                "encT": np.ascontiguousarray(e6[c]),
                "rnnT": np.ascontiguousarray(
                    rnn[sl].T.astype(bf).reshape(RC, 128, BL)
                ),
                "wrecT": wrecT,
                "packd": np.ascontiguousarray(packd),
            }
        )
    return in_maps


def kernel(
    encoded_contribution,
    mask,
    rnn_state,
    prev_att_weights,
    W_rec,
    w_score,
    b_score,
):
    from concourse.bass_utils import run_bass_kernel_spmd

    nc = _get_graph()
    in_maps = make_in_maps(
        np.asarray(encoded_contribution),
        np.asarray(mask),
        np.asarray(rnn_state),
        np.asarray(W_rec),
        np.asarray(w_score),
    )
    res = run_bass_kernel_spmd(nc, in_maps, list(range(NCORES)))
    outs = [np.asarray(res.results[c]["out"]) for c in range(NCORES)]
    return np.concatenate([o.T for o in outs], axis=1).astype(np.float32)
